# revision 4
# baseline (speedup 1.0000x reference)
"""BitNet-style quantized MLP (nn_ExpertMLP) on 8 Trainium2 NeuronCores.

Math (per reference):
    h = silu(Qa(x) @ Qw(W1).T);  y = Qa(h) @ Qw(W2).T
    Qa: per-token int8 absmax quant  -> round(x * 127/clip(max|x|,1e-5)) / s
    Qw: per-tensor ternary quant     -> clip(round(w / clip(mean|w|,1e-5)), -1, 1) * mean

Strategy: pure data parallel over tokens (2048 tokens/core, no collectives).
Quantized values are small integers, so matmuls run EXACTLY as bf16(acts) x
fp8e4(ternary weights) with fp32 PSUM accumulation; per-token/per-tensor
scales are folded into the output epilogues.

Two phases per core, ternary weights SBUF-resident as fp8 (16.8 MB each):
  A: stream+quantize W1^T once -> resident; per 128-token tile: quantize x
     (exact RNE via +1.5*2^23 magic add), DMA-transpose to xqT, 256 matmuls,
     silu+scale epilogue, quantize h, spill hq^T (bf16 ints, exact) to DRAM.
  B: stream+quantize W2^T once -> resident; per token tile: load hq^T slab,
     256 matmuls, scale epilogue -> y.

Host-side work is layout/sharding prep only (transpose W for contiguous DMA,
shard x) plus the two scalar weight-scale means.
"""
import numpy as np

import bass_rust
import concourse.bass as bass
import concourse.tile as tile
from concourse import mybir
from concourse.bass_utils import run_bass_kernel_spmd
from concourse.vector_clock import ScopedClock

D_MODEL = 2048
D_FF = 8192
N_CORES = 8
T_TOTAL = 4 * 4096
T_CORE = T_TOTAL // N_CORES          # 2048 tokens per core
N_TT = T_CORE // 128                 # 16 token tiles per core
N_DT = D_MODEL // 128                # 16 contraction tiles (layer 1)
N_FC = D_FF // 512                   # 16 f-chunks of 512
N_FT = D_FF // 128                   # 64 contraction tiles (layer 2)
N_MC = D_MODEL // 512                # 4 output chunks (layer 2)

MAGIC = 12582912.0                   # 1.5 * 2**23: (x + MAGIC) - MAGIC == RNE round(x)
F32 = mybir.dt.float32
BF16 = mybir.dt.bfloat16
FP8 = mybir.dt.float8e4

# ---------------------------------------------------------------------------
# walrus in this container rejects instructions carrying >1 sem wait
# ("Too many sync wait commands"); split excess waits onto same-engine NOPs.
MAXW = 1


def _split_one(nc, bb, inst):
    si = inst.sync_info
    waits = list(si.on_wait) if si and si.on_wait else []
    if len(waits) <= MAXW:
        return
    keep, extra = waits[-MAXW:], waits[:-MAXW]
    inst.sync_info = bass_rust.SyncInfo(on_wait=keep, on_update=list(si.on_update or []))
    eng = nc.engines[inst.engine]
    nops = []
    for i in range(0, len(extra), MAXW):
        n = eng.nop()
        n.ins.sync_info = bass_rust.SyncInfo(on_wait=extra[i:i + MAXW], on_update=[])
        nops.append(n.ins)
    cur = nc.cur_bb.bb
    cur_insts = cur.instructions
    for n in nops:
        for j in range(len(cur_insts) - 1, -1, -1):
            if cur_insts[j].name == n.name:
                cur_insts.pop(j)
                break
    cur.instructions = cur_insts
    insts = bb.instructions
    for j, x in enumerate(insts):
        if x.name == inst.name:
            for k, n in enumerate(nops):
                insts.insert(j + k, n)
            break
    bb.instructions = insts


def split_waits(nc):
    for _, bass_bb in list(nc.bb_map.items()):
        bb = bass_bb.bb
        for inst in list(bb.instructions):
            si = inst.sync_info
            if si and si.on_wait and len(si.on_wait) > MAXW:
                _split_one(nc, bb, inst)


class SplitDrainTC(tile.TileContext):
    def _drain_and_barrier(self, tick_clock, wait_clock):
        nc = self.nc
        probe = nc.sync.nop()
        wait_clock.add_sem_waits(probe.ins, ScopedClock({None: tick_clock.global_clock}))
        si = probe.ins.sync_info
        waits = list(si.on_wait) if si and si.on_wait else []
        if len(waits) > MAXW:
            probe.ins.sync_info = bass_rust.SyncInfo(
                on_wait=waits[:MAXW], on_update=list(si.on_update or []))
            for i in range(MAXW, len(waits), MAXW):
                n2 = nc.sync.nop()
                n2.ins.sync_info = bass_rust.SyncInfo(on_wait=waits[i:i + MAXW], on_update=[])
        nc.sync.drain()
        nc.all_engine_barrier()
        popped = nc._tile_sem_poison_stack.pop()
        assert popped is self._sem_poison
        nc.clear_and_free_semaphores(list(self.sems.allocated().values()))
        nc.all_engine_barrier()


# ---------------------------------------------------------------------------


def _quant_round(nc, pool, src_ap, scale_ap, magic_ap, out_tile, tag_tmp, shape):
    """out(bf16) = round(src*scale) done as ((src*scale + MAGIC) - MAGIC)."""
    t1 = pool.tile(shape, F32, tag=tag_tmp, name=f"qr_{tag_tmp}", bufs=2)
    nc.scalar.activation(out=t1, in_=src_ap, func=mybir.ActivationFunctionType.Identity,
                         bias=magic_ap, scale=scale_ap)
    nc.vector.tensor_scalar_add(out_tile, t1, -MAGIC)


def _build_nc():
    nc = bass.Bass()
    x_in = nc.declare_dram_parameter("x", [T_CORE, D_MODEL], F32, isOutput=False)
    w1t = nc.declare_dram_parameter("w1t", [D_MODEL, D_FF], F32, isOutput=False)
    w2t = nc.declare_dram_parameter("w2t", [D_FF, D_MODEL], F32, isOutput=False)
    # [s_w1, s_w2, a1mul=clip(mean|W1|)/127, a2mul=clip(mean|W2|)/127]
    consts = nc.declare_dram_parameter("consts", [1, 4], F32, isOutput=False)
    y_out = nc.declare_dram_parameter("y", [T_CORE, D_MODEL], F32, isOutput=True)

    AF = mybir.ActivationFunctionType

    with SplitDrainTC(nc) as tc:
        with tc.tile_pool(name="persist", bufs=1) as persist:
            csb = persist.tile([128, 4], F32, tag="consts")
            nc.sync.dma_start(out=csb, in_=consts[0:1, :].to_broadcast((128, 4)))
            s_w1 = csb[:, 0:1]
            s_w2 = csb[:, 1:2]
            a1mul = csb[:, 2:3]
            a2mul = csb[:, 3:4]
            magic = persist.tile([128, 1], F32, tag="magic")
            nc.vector.memset(magic, MAGIC)
            alpha2 = persist.tile([128, N_TT], F32, tag="alpha2")

            with tc.tile_pool(name="hspill", bufs=1, space="DRAM") as dpool:
                hsp = [dpool.tile([D_FF, 128], BF16, tag=f"hsp{tt}", name=f"hsp_{tt}") for tt in range(N_TT)]
                w2d = [dpool.tile([128, D_MODEL], FP8, tag=f"w2d{f}", name=f"w2d_{f}")
                       for f in range(N_FT)]

                # ============================= PHASE A =============================
                with tc.tile_pool(name="w1res", bufs=1) as w1pool:
                    w1q = [w1pool.tile([128, D_FF], FP8, tag=f"w1_{d}", name=f"w1q_{d}") for d in range(N_DT)]

                    with tc.tile_pool(name="a0", bufs=4) as a0:
                        # fc-major: phase-A matmuls consume f-chunks in this
                        # order, so the PE wavefront follows the stream.
                        for fc in range(N_FC):
                            fs = slice(fc * 512, (fc + 1) * 512)
                            for d in range(N_DT):
                                st = a0.tile([128, 512], F32, tag="w1st")
                                nc.sync.dma_start(out=st, in_=w1t[d * 128:(d + 1) * 128, fs])
                                t1 = a0.tile([128, 512], F32, tag="w1t1")
                                nc.scalar.activation(out=t1, in_=st, func=AF.Identity,
                                                     bias=magic, scale=s_w1)
                                nc.vector.tensor_scalar(
                                    out=t1, in0=t1, scalar1=-MAGIC, scalar2=1.0,
                                    op0=mybir.AluOpType.add, op1=mybir.AluOpType.min)
                                nc.vector.tensor_scalar(
                                    out=w1q[d][:, fs], in0=t1, scalar1=-1.0, scalar2=None,
                                    op0=mybir.AluOpType.max)

                    with tc.tile_pool(name="am", bufs=1) as am, \
                         tc.tile_pool(name="psA", bufs=8, space="PSUM") as psA:
                        for tt in range(N_TT):
                            trow = slice(tt * 128, (tt + 1) * 128)
                            # pass 1 over x: per-token absmax
                            cmax = am.tile([128, 4], F32, tag="cmax", bufs=2)
                            for c in range(4):
                                cs = slice(c * 512, (c + 1) * 512)
                                xst = am.tile([128, 512], F32, tag="xst", bufs=4)
                                nc.sync.dma_start(out=xst, in_=x_in[trow, cs])
                                nc.vector.tensor_reduce(
                                    out=cmax[:, c:c + 1], in_=xst, axis=mybir.AxisListType.X,
                                    op=mybir.AluOpType.max, apply_absolute_value=True)
                            amax = am.tile([128, 1], F32, tag="amax", bufs=2)
                            nc.vector.tensor_reduce(out=amax, in_=cmax, axis=mybir.AxisListType.X,
                                                    op=mybir.AluOpType.max)
                            nc.vector.tensor_scalar_max(amax, amax, 1e-5)
                            s1 = am.tile([128, 1], F32, tag="s1", bufs=2)
                            nc.vector.reciprocal(s1, amax)
                            nc.vector.tensor_scalar_mul(s1, s1, 127.0)
                            al1 = am.tile([128, 1], F32, tag="al1", bufs=2)
                            nc.vector.tensor_scalar(out=al1, in0=amax, scalar1=a1mul,
                                                    scalar2=None, op0=mybir.AluOpType.mult)
                            # pass 2 over x: quantize + transpose
                            xqT = am.tile([128, N_DT, 128], BF16, tag="xqT", bufs=2)
                            for c in range(4):
                                cs = slice(c * 512, (c + 1) * 512)
                                xst2 = am.tile([128, 512], F32, tag="xst", bufs=4)
                                nc.sync.dma_start(out=xst2, in_=x_in[trow, cs])
                                xqc = am.tile([128, 512], BF16, tag="xqc", bufs=2)
                                _quant_round(nc, am, xst2, s1, magic, xqc, "qt", [128, 512])
                                nc.sync.dma_start_transpose(xqT[:, c * 4:(c + 1) * 4, :], xqc)

                            # matmuls: h[t, f] += xqT[d].T @ w1q[d][:, fc]
                            hch = []
                            mh = am.tile([128, 1], F32, tag="mh", bufs=2)
                            for blk in range(4):
                                pss = []
                                for i in range(4):
                                    ps = psA.tile([128, 512], F32, tag="psA", name=f"psA_{tt}_{blk}_{i}")
                                    pss.append(ps)
                                for d in range(N_DT):
                                    for i in range(4):
                                        fc = blk * 4 + i
                                        nc.tensor.matmul(
                                            pss[i], lhsT=xqT[:, d, :],
                                            rhs=w1q[d][:, fc * 512:(fc + 1) * 512],
                                            start=(d == 0), stop=(d == N_DT - 1))
                                for i in range(4):
                                    fc = blk * 4 + i
                                    hc = am.tile([128, 512], F32, tag="h", bufs=18)
                                    nc.scalar.activation(out=hc, in_=pss[i], func=AF.Silu,
                                                         scale=al1)
                                    hch.append(hc)
                                    hm = am.tile([128, 1], F32, tag="hm", bufs=4)
                                    nc.vector.tensor_reduce(
                                        out=hm, in_=hc, axis=mybir.AxisListType.X,
                                        op=mybir.AluOpType.max, apply_absolute_value=True)
                                    if fc == 0:
                                        nc.vector.tensor_copy(mh, hm)
                                    else:
                                        nc.vector.tensor_tensor(
                                            out=mh, in0=mh, in1=hm, op=mybir.AluOpType.max)

                            nc.vector.tensor_scalar_max(mh, mh, 1e-5)
                            s2 = am.tile([128, 1], F32, tag="s2", bufs=2)
                            nc.vector.reciprocal(s2, mh)
                            nc.vector.tensor_scalar_mul(s2, s2, 127.0)
                            nc.vector.tensor_scalar(out=alpha2[:, tt:tt + 1], in0=mh,
                                                    scalar1=a2mul, scalar2=None,
                                                    op0=mybir.AluOpType.mult)
                            # quantize h, transpose, spill
                            for fc in range(N_FC):
                                hqc = am.tile([128, 512], BF16, tag="hqc", bufs=2)
                                _quant_round(nc, am, hch[fc], s2, magic, hqc, "qt", [128, 512])
                                hqtr = am.tile([128, 4, 128], BF16, tag="hqtr", bufs=2)
                                nc.sync.dma_start_transpose(hqtr, hqc)
                                nc.sync.dma_start(
                                    out=hsp[tt][fc * 512:(fc + 1) * 512, :].rearrange(
                                        "(b s) t -> s b t", b=4),
                                    in_=hqtr)

                            # prestage 16 W2 chunks -> fp8 in DRAM (overlapped
                            # under this tile's matmuls; phase B then streams
                            # fp8 directly, 4x less DMA at the phase boundary)
                            for j in range(16):
                                f2, mc2 = divmod(tt * 16 + j, 4)
                                ms2 = slice(mc2 * 512, (mc2 + 1) * 512)
                                st2 = am.tile([128, 512], F32, tag="w2st", bufs=2)
                                nc.sync.dma_start(
                                    out=st2, in_=w2t[f2 * 128:(f2 + 1) * 128, ms2])
                                t2 = am.tile([128, 512], F32, tag="qt", bufs=2)
                                nc.scalar.activation(out=t2, in_=st2, func=AF.Identity,
                                                     bias=magic, scale=s_w2)
                                nc.vector.tensor_scalar(
                                    out=t2, in0=t2, scalar1=-MAGIC, scalar2=1.0,
                                    op0=mybir.AluOpType.add, op1=mybir.AluOpType.min)
                                c8 = am.tile([128, 512], FP8, tag="w2c8", bufs=2)
                                nc.vector.tensor_scalar(
                                    out=c8, in0=t2, scalar1=-1.0, scalar2=None,
                                    op0=mybir.AluOpType.max)
                                nc.sync.dma_start(out=w2d[f2][:, ms2], in_=c8)

                # ============================= PHASE B =============================
                with tc.tile_pool(name="w2res", bufs=1) as w2pool:
                    w2q = [w2pool.tile([128, D_MODEL], FP8, tag=f"w2_{f}", name=f"w2q_{f}") for f in range(N_FT)]

                    for f in range(N_FT):
                        nc.sync.dma_start(out=w2q[f], in_=w2d[f][:])

                    with tc.tile_pool(name="bm", bufs=1) as bm, \
                         tc.tile_pool(name="psB", bufs=8, space="PSUM") as psB:
                        for tt in range(N_TT):
                            trow = slice(tt * 128, (tt + 1) * 128)
                            hslab = bm.tile([128, N_FT, 128], BF16, tag="hslab", bufs=2)
                            nc.sync.dma_start(
                                out=hslab, in_=hsp[tt].rearrange("(ft s) t -> s ft t", s=128))
                            pss = []
                            for mc in range(N_MC):
                                ps = psB.tile([128, 512], F32, tag="psB", name=f"psB_{tt}_{mc}")
                                pss.append(ps)
                            for f in range(N_FT):
                                for mc in range(N_MC):
                                    nc.tensor.matmul(
                                        pss[mc], lhsT=hslab[:, f, :],
                                        rhs=w2q[f][:, mc * 512:(mc + 1) * 512],
                                        start=(f == 0), stop=(f == N_FT - 1))
                            for mc in range(N_MC):
                                yc = bm.tile([128, 512], F32, tag="yc", bufs=4)
                                nc.scalar.activation(out=yc, in_=pss[mc], func=AF.Copy,
                                                     scale=alpha2[:, tt:tt + 1])
                                nc.sync.dma_start(
                                    out=y_out[trow, mc * 512:(mc + 1) * 512], in_=yc)

    split_waits(nc)
    return nc


_NC_CACHE = None


def _get_nc():
    global _NC_CACHE
    if _NC_CACHE is None:
        _NC_CACHE = _build_nc()
    return _NC_CACHE


def kernel(x, W1, W2):
    assert x.shape == (4, 4096, D_MODEL) and x.dtype == np.float32
    assert W1.shape == (D_FF, D_MODEL) and W2.shape == (D_MODEL, D_FF)

    x2d = np.ascontiguousarray(x.reshape(T_TOTAL, D_MODEL))
    w1t = np.ascontiguousarray(W1.T)            # [D_MODEL, D_FF]
    w2t = np.ascontiguousarray(W2.T)            # [D_FF, D_MODEL]

    m1 = max(float(np.mean(np.abs(W1), dtype=np.float32)), 1e-5)
    m2 = max(float(np.mean(np.abs(W2), dtype=np.float32)), 1e-5)
    consts = np.array([[1.0 / m1, 1.0 / m2, m1 / 127.0, m2 / 127.0]], dtype=np.float32)

    nc = _get_nc()
    in_maps = [
        {"x": x2d[c * T_CORE:(c + 1) * T_CORE], "w1t": w1t, "w2t": w2t, "consts": consts}
        for c in range(N_CORES)
    ]
    res = run_bass_kernel_spmd(nc, in_maps, list(range(N_CORES)), trace=False)
    y = np.concatenate([res.results[c]["y"] for c in range(N_CORES)], axis=0)
    return y.reshape(4, 4096, D_MODEL)


# revision 5
# speedup vs baseline: 1.0008x; 1.0008x over previous
"""BitNet-style quantized MLP (nn_ExpertMLP) on 8 Trainium2 NeuronCores.

Math (per reference):
    h = silu(Qa(x) @ Qw(W1).T);  y = Qa(h) @ Qw(W2).T
    Qa: per-token int8 absmax quant  -> round(x * 127/clip(max|x|,1e-5)) / s
    Qw: per-tensor ternary quant     -> clip(round(w / clip(mean|w|,1e-5)), -1, 1) * mean

Strategy: pure data parallel over tokens (2048 tokens/core, no collectives).
Quantized values are small integers, so matmuls run EXACTLY as bf16(acts) x
fp8e4(ternary weights) with fp32 PSUM accumulation; per-token/per-tensor
scales are folded into the output epilogues.

Two phases per core, ternary weights SBUF-resident as fp8 (16.8 MB each):
  A: stream+quantize W1^T once -> resident; per 128-token tile: quantize x
     (exact RNE via +1.5*2^23 magic add), DMA-transpose to xqT, 256 matmuls,
     silu+scale epilogue, quantize h, spill hq^T (bf16 ints, exact) to DRAM.
  B: stream+quantize W2^T once -> resident; per token tile: load hq^T slab,
     256 matmuls, scale epilogue -> y.

Host-side work is layout/sharding prep only (transpose W for contiguous DMA,
shard x) plus the two scalar weight-scale means.
"""
import numpy as np

import bass_rust
import concourse.bass as bass
import concourse.tile as tile
from concourse import mybir
from concourse.bass_utils import run_bass_kernel_spmd
from concourse.vector_clock import ScopedClock

D_MODEL = 2048
D_FF = 8192
N_CORES = 8
T_TOTAL = 4 * 4096
T_CORE = T_TOTAL // N_CORES          # 2048 tokens per core
N_TT = T_CORE // 128                 # 16 token tiles per core
N_DT = D_MODEL // 128                # 16 contraction tiles (layer 1)
N_FC = D_FF // 512                   # 16 f-chunks of 512
N_FT = D_FF // 128                   # 64 contraction tiles (layer 2)
N_MC = D_MODEL // 512                # 4 output chunks (layer 2)

MAGIC = 12582912.0                   # 1.5 * 2**23: (x + MAGIC) - MAGIC == RNE round(x)
F32 = mybir.dt.float32
BF16 = mybir.dt.bfloat16
FP8 = mybir.dt.float8e4

# ---------------------------------------------------------------------------
# walrus in this container rejects instructions carrying >1 sem wait
# ("Too many sync wait commands"); split excess waits onto same-engine NOPs.
MAXW = 1


def _split_one(nc, bb, inst):
    si = inst.sync_info
    waits = list(si.on_wait) if si and si.on_wait else []
    if len(waits) <= MAXW:
        return
    keep, extra = waits[-MAXW:], waits[:-MAXW]
    inst.sync_info = bass_rust.SyncInfo(on_wait=keep, on_update=list(si.on_update or []))
    eng = nc.engines[inst.engine]
    nops = []
    for i in range(0, len(extra), MAXW):
        n = eng.nop()
        n.ins.sync_info = bass_rust.SyncInfo(on_wait=extra[i:i + MAXW], on_update=[])
        nops.append(n.ins)
    cur = nc.cur_bb.bb
    cur_insts = cur.instructions
    for n in nops:
        for j in range(len(cur_insts) - 1, -1, -1):
            if cur_insts[j].name == n.name:
                cur_insts.pop(j)
                break
    cur.instructions = cur_insts
    insts = bb.instructions
    for j, x in enumerate(insts):
        if x.name == inst.name:
            for k, n in enumerate(nops):
                insts.insert(j + k, n)
            break
    bb.instructions = insts


def split_waits(nc):
    for _, bass_bb in list(nc.bb_map.items()):
        bb = bass_bb.bb
        for inst in list(bb.instructions):
            si = inst.sync_info
            if si and si.on_wait and len(si.on_wait) > MAXW:
                _split_one(nc, bb, inst)


class SplitDrainTC(tile.TileContext):
    def _drain_and_barrier(self, tick_clock, wait_clock):
        nc = self.nc
        probe = nc.sync.nop()
        wait_clock.add_sem_waits(probe.ins, ScopedClock({None: tick_clock.global_clock}))
        si = probe.ins.sync_info
        waits = list(si.on_wait) if si and si.on_wait else []
        if len(waits) > MAXW:
            probe.ins.sync_info = bass_rust.SyncInfo(
                on_wait=waits[:MAXW], on_update=list(si.on_update or []))
            for i in range(MAXW, len(waits), MAXW):
                n2 = nc.sync.nop()
                n2.ins.sync_info = bass_rust.SyncInfo(on_wait=waits[i:i + MAXW], on_update=[])
        nc.sync.drain()
        nc.all_engine_barrier()
        popped = nc._tile_sem_poison_stack.pop()
        assert popped is self._sem_poison
        nc.clear_and_free_semaphores(list(self.sems.allocated().values()))
        nc.all_engine_barrier()


# ---------------------------------------------------------------------------


def _quant_round(nc, pool, src_ap, scale_ap, magic_ap, out_tile, tag_tmp, shape):
    """out(bf16) = round(src*scale) done as ((src*scale + MAGIC) - MAGIC)."""
    t1 = pool.tile(shape, F32, tag=tag_tmp, name=f"qr_{tag_tmp}", bufs=2)
    nc.scalar.activation(out=t1, in_=src_ap, func=mybir.ActivationFunctionType.Identity,
                         bias=magic_ap, scale=scale_ap)
    nc.vector.tensor_scalar_add(out_tile, t1, -MAGIC)


def _build_nc():
    nc = bass.Bass()
    x_in = nc.declare_dram_parameter("x", [T_CORE, D_MODEL], F32, isOutput=False)
    w1t = nc.declare_dram_parameter("w1t", [D_MODEL, D_FF], F32, isOutput=False)
    w2t = nc.declare_dram_parameter("w2t", [D_FF, D_MODEL], F32, isOutput=False)
    # [s_w1, s_w2, a1mul=clip(mean|W1|)/127, a2mul=clip(mean|W2|)/127]
    consts = nc.declare_dram_parameter("consts", [1, 4], F32, isOutput=False)
    y_out = nc.declare_dram_parameter("y", [T_CORE, D_MODEL], F32, isOutput=True)

    AF = mybir.ActivationFunctionType

    with SplitDrainTC(nc) as tc:
        with tc.tile_pool(name="persist", bufs=1) as persist:
            csb = persist.tile([128, 4], F32, tag="consts")
            nc.sync.dma_start(out=csb, in_=consts[0:1, :].to_broadcast((128, 4)))
            s_w1 = csb[:, 0:1]
            s_w2 = csb[:, 1:2]
            a1mul = csb[:, 2:3]
            a2mul = csb[:, 3:4]
            magic = persist.tile([128, 1], F32, tag="magic")
            nc.vector.memset(magic, MAGIC)
            alpha2 = persist.tile([128, N_TT], F32, tag="alpha2")

            with tc.tile_pool(name="hspill", bufs=1, space="DRAM") as dpool:
                hsp = [dpool.tile([D_FF, 128], BF16, tag=f"hsp{tt}", name=f"hsp_{tt}") for tt in range(N_TT)]
                w2d = [dpool.tile([128, D_MODEL], FP8, tag=f"w2d{f}", name=f"w2d_{f}")
                       for f in range(N_FT)]

                # ============================= PHASE A =============================
                with tc.tile_pool(name="w1res", bufs=1) as w1pool:
                    w1q = [w1pool.tile([128, D_FF], FP8, tag=f"w1_{d}", name=f"w1q_{d}") for d in range(N_DT)]

                    with tc.tile_pool(name="a0", bufs=6) as a0:
                        # fc-major: phase-A matmuls consume f-chunks in this
                        # order, so the PE wavefront follows the stream.
                        for fc in range(N_FC):
                            fs = slice(fc * 512, (fc + 1) * 512)
                            for d in range(N_DT):
                                st = a0.tile([128, 512], F32, tag="w1st")
                                nc.sync.dma_start(out=st, in_=w1t[d * 128:(d + 1) * 128, fs])
                                t1 = a0.tile([128, 512], F32, tag="w1t1")
                                nc.scalar.activation(out=t1, in_=st, func=AF.Identity,
                                                     bias=magic, scale=s_w1)
                                nc.vector.tensor_scalar(
                                    out=t1, in0=t1, scalar1=-MAGIC, scalar2=1.0,
                                    op0=mybir.AluOpType.add, op1=mybir.AluOpType.min)
                                nc.vector.tensor_scalar(
                                    out=w1q[d][:, fs], in0=t1, scalar1=-1.0, scalar2=None,
                                    op0=mybir.AluOpType.max)

                    with tc.tile_pool(name="am", bufs=1) as am, \
                         tc.tile_pool(name="psA", bufs=8, space="PSUM") as psA:
                        for tt in range(N_TT):
                            trow = slice(tt * 128, (tt + 1) * 128)
                            # pass 1 over x: per-token absmax
                            cmax = am.tile([128, 4], F32, tag="cmax", bufs=2)
                            for c in range(4):
                                cs = slice(c * 512, (c + 1) * 512)
                                xst = am.tile([128, 512], F32, tag="xst", bufs=4)
                                nc.scalar.dma_start(out=xst, in_=x_in[trow, cs])
                                nc.vector.tensor_reduce(
                                    out=cmax[:, c:c + 1], in_=xst, axis=mybir.AxisListType.X,
                                    op=mybir.AluOpType.max, apply_absolute_value=True)
                            amax = am.tile([128, 1], F32, tag="amax", bufs=2)
                            nc.vector.tensor_reduce(out=amax, in_=cmax, axis=mybir.AxisListType.X,
                                                    op=mybir.AluOpType.max)
                            nc.vector.tensor_scalar_max(amax, amax, 1e-5)
                            s1 = am.tile([128, 1], F32, tag="s1", bufs=2)
                            nc.vector.reciprocal(s1, amax)
                            nc.vector.tensor_scalar_mul(s1, s1, 127.0)
                            al1 = am.tile([128, 1], F32, tag="al1", bufs=2)
                            nc.vector.tensor_scalar(out=al1, in0=amax, scalar1=a1mul,
                                                    scalar2=None, op0=mybir.AluOpType.mult)
                            # pass 2 over x: quantize + transpose
                            xqT = am.tile([128, N_DT, 128], BF16, tag="xqT", bufs=2)
                            for c in range(4):
                                cs = slice(c * 512, (c + 1) * 512)
                                xst2 = am.tile([128, 512], F32, tag="xst", bufs=4)
                                nc.scalar.dma_start(out=xst2, in_=x_in[trow, cs])
                                xqc = am.tile([128, 512], BF16, tag="xqc", bufs=2)
                                _quant_round(nc, am, xst2, s1, magic, xqc, "qt", [128, 512])
                                nc.scalar.dma_start_transpose(xqT[:, c * 4:(c + 1) * 4, :], xqc)

                            # matmuls: h[t, f] += xqT[d].T @ w1q[d][:, fc]
                            hch = []
                            mh = am.tile([128, 1], F32, tag="mh", bufs=2)
                            for blk in range(4):
                                pss = []
                                for i in range(4):
                                    ps = psA.tile([128, 512], F32, tag="psA", name=f"psA_{tt}_{blk}_{i}")
                                    pss.append(ps)
                                for d in range(N_DT):
                                    for i in range(4):
                                        fc = blk * 4 + i
                                        nc.tensor.matmul(
                                            pss[i], lhsT=xqT[:, d, :],
                                            rhs=w1q[d][:, fc * 512:(fc + 1) * 512],
                                            start=(d == 0), stop=(d == N_DT - 1))
                                for i in range(4):
                                    fc = blk * 4 + i
                                    hc = am.tile([128, 512], F32, tag="h", bufs=18)
                                    nc.scalar.activation(out=hc, in_=pss[i], func=AF.Silu,
                                                         scale=al1)
                                    hch.append(hc)
                                    hm = am.tile([128, 1], F32, tag="hm", bufs=4)
                                    nc.vector.tensor_reduce(
                                        out=hm, in_=hc, axis=mybir.AxisListType.X,
                                        op=mybir.AluOpType.max, apply_absolute_value=True)
                                    if fc == 0:
                                        nc.vector.tensor_copy(mh, hm)
                                    else:
                                        nc.vector.tensor_tensor(
                                            out=mh, in0=mh, in1=hm, op=mybir.AluOpType.max)

                            nc.vector.tensor_scalar_max(mh, mh, 1e-5)
                            s2 = am.tile([128, 1], F32, tag="s2", bufs=2)
                            nc.vector.reciprocal(s2, mh)
                            nc.vector.tensor_scalar_mul(s2, s2, 127.0)
                            nc.vector.tensor_scalar(out=alpha2[:, tt:tt + 1], in0=mh,
                                                    scalar1=a2mul, scalar2=None,
                                                    op0=mybir.AluOpType.mult)
                            # quantize h, transpose, spill
                            for fc in range(N_FC):
                                hqc = am.tile([128, 512], BF16, tag="hqc", bufs=2)
                                _quant_round(nc, am, hch[fc], s2, magic, hqc, "qt", [128, 512])
                                hqtr = am.tile([128, 4, 128], BF16, tag="hqtr", bufs=2)
                                nc.sync.dma_start_transpose(hqtr, hqc)
                                nc.sync.dma_start(
                                    out=hsp[tt][fc * 512:(fc + 1) * 512, :].rearrange(
                                        "(b s) t -> s b t", b=4),
                                    in_=hqtr)

                            # prestage 16 W2 chunks -> fp8 in DRAM (overlapped
                            # under this tile's matmuls; phase B then streams
                            # fp8 directly, 4x less DMA at the phase boundary)
                            for j in range(16):
                                f2, mc2 = divmod(tt * 16 + j, 4)
                                ms2 = slice(mc2 * 512, (mc2 + 1) * 512)
                                st2 = am.tile([128, 512], F32, tag="w2st", bufs=2)
                                nc.sync.dma_start(
                                    out=st2, in_=w2t[f2 * 128:(f2 + 1) * 128, ms2])
                                t2 = am.tile([128, 512], F32, tag="qt", bufs=2)
                                nc.scalar.activation(out=t2, in_=st2, func=AF.Identity,
                                                     bias=magic, scale=s_w2)
                                nc.vector.tensor_scalar(
                                    out=t2, in0=t2, scalar1=-MAGIC, scalar2=1.0,
                                    op0=mybir.AluOpType.add, op1=mybir.AluOpType.min)
                                c8 = am.tile([128, 512], FP8, tag="w2c8", bufs=2)
                                nc.vector.tensor_scalar(
                                    out=c8, in0=t2, scalar1=-1.0, scalar2=None,
                                    op0=mybir.AluOpType.max)
                                nc.sync.dma_start(out=w2d[f2][:, ms2], in_=c8)

                # ============================= PHASE B =============================
                with tc.tile_pool(name="w2res", bufs=1) as w2pool:
                    w2q = [w2pool.tile([128, D_MODEL], FP8, tag=f"w2_{f}", name=f"w2q_{f}") for f in range(N_FT)]

                    for f in range(N_FT):
                        nc.sync.dma_start(out=w2q[f], in_=w2d[f][:])

                    with tc.tile_pool(name="bm", bufs=1) as bm, \
                         tc.tile_pool(name="psB", bufs=8, space="PSUM") as psB:
                        for tt in range(N_TT):
                            trow = slice(tt * 128, (tt + 1) * 128)
                            hslab = bm.tile([128, N_FT, 128], BF16, tag="hslab", bufs=2)
                            nc.scalar.dma_start(
                                out=hslab, in_=hsp[tt].rearrange("(ft s) t -> s ft t", s=128))
                            pss = []
                            for mc in range(N_MC):
                                ps = psB.tile([128, 512], F32, tag="psB", name=f"psB_{tt}_{mc}")
                                pss.append(ps)
                            for f in range(N_FT):
                                for mc in range(N_MC):
                                    nc.tensor.matmul(
                                        pss[mc], lhsT=hslab[:, f, :],
                                        rhs=w2q[f][:, mc * 512:(mc + 1) * 512],
                                        start=(f == 0), stop=(f == N_FT - 1))
                            for mc in range(N_MC):
                                yc = bm.tile([128, 512], F32, tag="yc", bufs=4)
                                nc.scalar.activation(out=yc, in_=pss[mc], func=AF.Copy,
                                                     scale=alpha2[:, tt:tt + 1])
                                nc.sync.dma_start(
                                    out=y_out[trow, mc * 512:(mc + 1) * 512], in_=yc)

    split_waits(nc)
    return nc


_NC_CACHE = None


def _get_nc():
    global _NC_CACHE
    if _NC_CACHE is None:
        _NC_CACHE = _build_nc()
    return _NC_CACHE


def kernel(x, W1, W2):
    assert x.shape == (4, 4096, D_MODEL) and x.dtype == np.float32
    assert W1.shape == (D_FF, D_MODEL) and W2.shape == (D_MODEL, D_FF)

    x2d = np.ascontiguousarray(x.reshape(T_TOTAL, D_MODEL))
    w1t = np.ascontiguousarray(W1.T)            # [D_MODEL, D_FF]
    w2t = np.ascontiguousarray(W2.T)            # [D_FF, D_MODEL]

    m1 = max(float(np.mean(np.abs(W1), dtype=np.float32)), 1e-5)
    m2 = max(float(np.mean(np.abs(W2), dtype=np.float32)), 1e-5)
    consts = np.array([[1.0 / m1, 1.0 / m2, m1 / 127.0, m2 / 127.0]], dtype=np.float32)

    nc = _get_nc()
    in_maps = [
        {"x": x2d[c * T_CORE:(c + 1) * T_CORE], "w1t": w1t, "w2t": w2t, "consts": consts}
        for c in range(N_CORES)
    ]
    res = run_bass_kernel_spmd(nc, in_maps, list(range(N_CORES)), trace=False)
    y = np.concatenate([res.results[c]["y"] for c in range(N_CORES)], axis=0)
    return y.reshape(4, 4096, D_MODEL)
